# revision 30
# baseline (speedup 1.0000x reference)
"""Trainium2 Bass kernel for multi-head quadratic spatial attention.

Problem: q,k,v [b=8, heads=8, h=32, w=32, d=64] fp32; full attention over
the 1024-position spatial grid independently per (b, head); output
[b, h, w, heads*d].

Sharding: data-parallel over batch — core c handles b=c (8 heads of
[1024, 64] attention per core), no cross-core communication.

Per-core pipeline (heads processed in PAIRS; matmuls bf16 with fp32 PSUM
accumulation). The PE executes serially on this toolchain, so the design
minimizes streamed columns + instruction count and keeps the HAM clock
gate warm (no transpose-heavy stretches > ~3.4us, dummy-matmul warm-up):
  - p-major seq tiling (seq = p*8 + t); ONE 4D casting DMA per (tensor,
    pair) interleaving the two heads -> 3 gpsimd triggers per pair
  - 40 dummy ident matmuls warm the PE clock gate (1.2 -> 2.4 GHz) while
    the first DMAs land
  - pair-interleaved natural tiles [128, t, 2, d]: one [128,128] PE
    transpose per block yields head A's d-rows on partitions 0:64 and
    B's on 64:128 — the packed pair layout mm1 wants
  - mm1 row-tiled: head A contracts on PE rows 0:64, head B on 64:128
    -> St [128, 1024] fp32 (separate tiles, freed by their own exp)
  - exp on ScalarE (activation Exp); optional per-(jb, head) offload to
    VectorE via the Schraudolph bit-trick (fused tensor_scalar
    mult+add -> int16 == bf16 exp approx) to unload the ScalarE
  - mm2: lhsT = [V | 1] j-chunk [128, 65] bf16, rhs = Pt slices ->
    accumulate PSUM Ot [65, 512] per i-half; row 64 = softmax sums
  - epilogue in bf16: ot copy on VectorE, PE transposes back (FWL), one
    batched reciprocal [128,4] + per-block tensor_scalar normalize into
    fp32 ostage, stores on the sync HWDGE ring
"""

from contextlib import ExitStack

import numpy as np

F32 = None
BF16 = None
I16 = None

_cache = {}

# Schraudolph exp in bf16 bit-space: bf16_bits(exp(s*x)) ~= round(x*A + B)
# A = s * 2^7/ln2, B = 2^7*(127 - sigma), sigma = 0.0430 balances the
# piecewise-linear 2^frac error (max rel err ~3%, mostly cancelling in the
# softmax average).
SCALE = 64.0 ** -0.5
SCHRAUD_A = SCALE * 128.0 / float(np.log(2.0))
SCHRAUD_B = 128.0 * (127.0 - 0.0430)

N_WARM = 10  # dummy matmuls to flip the PE HAM clock gate before real work

# (jb, head-in-pair) St tiles exp'd on VectorE via the Schraudolph bit-trick
# instead of ScalarE's exact exp. Head B's tiles all go to VectorE: exp-A
# (ScalarE, ~1.15us) and exp-B (VectorE, ~1.19us) then run CONCURRENTLY, so
# stA/stB release near-simultaneously and the scheduler can bake the mm1
# quartet as A,B,A,B (adjacent disjoint row groups -> concurrent on the PE).
SCHRAUD_TILES = frozenset((jb, 1) for jb in range(8))


def _imports():
    global F32, BF16, I16
    import concourse.bass as bass
    import concourse.tile as tile
    from concourse import mybir
    from concourse.masks import make_identity

    F32 = mybir.dt.float32
    BF16 = mybir.dt.bfloat16
    I16 = mybir.dt.int16
    return bass, tile, mybir, make_identity


def _interleave_quartets(nc, mybir, quartets):
    """Post-schedule pass: rebake each mm1 quartet as A0,B0,A1,B1 in the PE
    stream (adjacent disjoint row groups stream CONCURRENTLY on the PE).

    The tile scheduler's cost model sees the DVE as backlogged and bakes the
    B matmuls several mm2 slots after the A's, which forfeits row-group
    concurrency.  On hardware the B's DVE wait is long satisfied by then, so
    we permute the PE stream directly: pull each B (with its LDWEIGHTS)
    forward to sit right after its A sibling.  All waits (any engine) on
    semaphores that PE instructions increment are then remapped: a wait for
    "first v PE-increments done" must now cover the same SET of original
    instructions at their new positions, i.e. new_v = prefix count at the
    maximum new position of that set.
    """
    pe_eng = mybir.EngineType.PE

    for f in nc.m.functions:
        for bb in f.blocks:
            insts = bb.instructions
            pe_idx = [i for i, ins in enumerate(insts) if ins.engine == pe_eng]
            if not pe_idx:
                continue
            stream = [insts[i] for i in pe_idx]
            name_pos = {ins.name: p for p, ins in enumerate(stream)}

            # Pair each matmul with ITS ldweights by weights-AP content: the
            # PE reorder window means the baked stream can hoist an ldw
            # several slots ahead, so adjacency is not a safe pairing rule
            # (moving a matmul without its ldw reorders the weight load
            # after the matmul -> wrong weights / wedged device).
            def w_sig(ins):
                tn = type(ins).__name__
                if tn == "InstLdweights":
                    return str(ins.ins[0])
                if tn == "InstMatmult":
                    return str(ins.ins[1])
                return None

            claimed_ldw = set()

            def unit(p):
                sig = w_sig(stream[p])
                for q in range(p - 1, max(-1, p - 8), -1):
                    if (q not in claimed_ldw
                            and type(stream[q]).__name__ == "InstLdweights"
                            and w_sig(stream[q]) == sig):
                        claimed_ldw.add(q)
                        return [q, p]
                return [p]

            order = list(range(len(stream)))
            for qa0, qb0, qa1, qb1 in quartets:
                if any(n not in name_pos for n in (qa0, qb0, qa1, qb1)):
                    continue
                # desired: A0 < B0 < A1 < B1 (each with its ldw) contiguous
                units = [unit(name_pos[n]) for n in (qa0, qb0, qa1, qb1)]
                taken = [t for u in units for t in u]
                taken_set = set(taken)
                anchor = min(order.index(t) for t in taken)
                rest = [x for x in order if x not in taken_set]
                order = rest[:anchor] + taken + rest[anchor:]

            if order == list(range(len(stream))):
                continue

            new_stream = [stream[i] for i in order]
            # old position -> new position
            new_pos_of_old = [0] * len(stream)
            for newp, oldp in enumerate(order):
                new_pos_of_old[oldp] = newp

            # per-sem prefix increment arrays (old and new order)
            def upd_map(ins):
                out = {}
                si = ins.sync_info
                if si and si.on_update:
                    for u in si.on_update:
                        if u.sync_type == "semaphore":
                            out[u.id] = out.get(u.id, 0) + (u.update_value or 1)
                return out

            sem_ids = set()
            for ins in stream:
                sem_ids.update(upd_map(ins))
            old_prefix = {s: [0] * (len(stream) + 1) for s in sem_ids}
            new_prefix = {s: [0] * (len(stream) + 1) for s in sem_ids}
            for p, ins in enumerate(stream):
                um = upd_map(ins)
                for s in sem_ids:
                    old_prefix[s][p + 1] = old_prefix[s][p] + um.get(s, 0)
            for p, ins in enumerate(new_stream):
                um = upd_map(ins)
                for s in sem_ids:
                    new_prefix[s][p + 1] = new_prefix[s][p] + um.get(s, 0)

            def remap_wait(w):
                if w.sync_type != "semaphore" or w.id not in sem_ids:
                    return
                v = w.wait_value
                if v is None or v <= 0:
                    return
                op = old_prefix[w.id]
                if v > op[-1]:
                    return  # counts from a previous block epoch etc.
                # minimal k with old_prefix[k] >= v  -> set = old stream [0:k)
                import bisect
                k = bisect.bisect_left(op, v)
                max_new = max(new_pos_of_old[i] for i in range(k))
                w.wait_value = new_prefix[w.id][max_new + 1]

            for bb2 in f.blocks:
                for ins in bb2.instructions:
                    si = ins.sync_info
                    if si and si.on_wait:
                        for w in si.on_wait:
                            remap_wait(w)

            # write back: permuted PE stream into the same slots
            for slot, ins in zip(pe_idx, new_stream):
                insts[slot] = ins
            bb.instructions = insts


def _split_multi_waits(nc, mybir):
    """Walrus in this container supports only ONE sync-wait per instruction.
    Hoist extra waits onto same-engine InstNoOp's inserted just before."""
    ctr = 0
    for f in nc.m.functions:
        for bb in f.blocks:
            insts = bb.instructions
            if not any(
                i.sync_info and i.sync_info.on_wait and len(i.sync_info.on_wait) > 1
                for i in insts
            ):
                continue
            out = []
            for inst in insts:
                si = inst.sync_info
                waits = list(si.on_wait) if si and si.on_wait else []
                if len(waits) > 1:
                    for w in waits[:-1]:
                        ctr += 1
                        nop = mybir.InstNoOp(
                            name=f"I-wsplit-{ctr}",
                            engine=inst.engine,
                            ins=[],
                            outs=[],
                            sync_info=mybir.SyncInfo(on_wait=[w], on_update=[]),
                        )
                        nc.register_instruction(nop)
                        out.append(nop)
                    si.on_wait = waits[-1:]
                out.append(inst)
            bb.instructions = out


def _build_nc(heads=8, seq=1024, d=64):
    bass, tile, mybir, make_identity = _imports()
    assert heads % 2 == 0 and seq == 1024 and d == 64
    nt = seq // 128          # 8 blocks of 128 positions
    nh = seq // 512          # 2 i-halves of 512
    dv = d + 1
    TS_MULT = mybir.AluOpType.mult
    TS_ADD = mybir.AluOpType.add

    nc = bass.Bass(trn_type="TRN2", target_bir_lowering=False)
    quartets = []  # (A0, B0, A1, B1) matmul names for post-schedule interleave
    q_d = nc.dram_tensor("q", [heads, seq, d], F32, kind="ExternalInput")
    k_d = nc.dram_tensor("k", [heads, seq, d], F32, kind="ExternalInput")
    v_d = nc.dram_tensor("v", [heads, seq, d], F32, kind="ExternalInput")
    o_d = nc.dram_tensor("out", [seq, heads * d], F32, kind="ExternalOutput")

    # p-major: seq = p*nt + t; per-(p, t) HBM runs are 256B contiguous
    q_ap = q_d[:].rearrange("n (p t) d -> n p t d", p=128)
    k_ap = k_d[:].rearrange("n (p t) d -> n p t d", p=128)
    v_ap = v_d[:].rearrange("n (p t) d -> n p t d", p=128)
    o_ap = o_d[:].rearrange("(p t) c -> p t c", p=128)

    with tile.TileContext(nc) as tc, ExitStack() as ctx:
        consts = ctx.enter_context(tc.tile_pool(name="consts", bufs=1))
        nat = ctx.enter_context(tc.tile_pool(name="nat", bufs=2))
        dmaj = ctx.enter_context(tc.tile_pool(name="dmaj", bufs=2))
        ptp = ctx.enter_context(tc.tile_pool(name="ptp", bufs=36))
        outp = ctx.enter_context(tc.tile_pool(name="outp", bufs=3))
        small = ctx.enter_context(tc.tile_pool(name="small", bufs=4))

        # PSUM banks: st 2x2 (0-3) + oacc/ob/warm 2x1 (4-5) + tp 2x1 (6-7)
        st_ps = ctx.enter_context(tc.tile_pool(name="st_ps", bufs=2, space="PSUM"))
        oa_ps = ctx.enter_context(tc.tile_pool(name="oa_ps", bufs=2, space="PSUM"))
        tp_ps = ctx.enter_context(tc.tile_pool(name="tp_ps", bufs=2, space="PSUM"))

        ident_bf = consts.tile([128, 128], BF16)
        make_identity(nc, ident_bf[:])

        # Warm-up / filler matmuls keep the PE HAM clock gate at 2.4 GHz:
        # an idle (or transpose-only) stretch > ~3.4us re-throttles the PE
        # clock to 1.2 GHz for the next several microseconds. wsrc is
        # memset-ready within ~200ns of kernel start. N=512 streams give
        # ~213ns of HAM-counted busy per filler instruction.
        wsrc = consts.tile([128, 512], BF16)
        nc.vector.memset(wsrc[:], 0.25)

        def pe_filler(n):
            # fresh tile per burst: fillers WAR-chain only onto transient
            # transpose tiles, never onto live oacc accumulators
            t = tp_ps.tile([128, 512], F32, tag="tp", name="warm")
            for _ in range(n):
                nc.tensor.matmul(
                    t[:], wsrc[:, 0:128], wsrc[:], start=True, stop=True
                )

        pe_filler(N_WARM)

        def load_and_transpose(pair):
            """DMA pair inputs (bf16 cast, one 4D DMA per tensor) and build
            packed d-major tiles: head A on partitions 0:64, head B on
            64:128 (one [128,128] PE transpose per block)."""
            st8 = {"heads": (2 * pair, 2 * pair + 1), "v": None, "pts": {},
                   "oacc": {}, "ostage": {}}
            # pair-interleaved natural tiles: [..., 2, d] with head A at
            # index 0 and head B at 1, so one [128, 128] PE transpose of a
            # block yields A's d-rows on partitions 0:64 and B's on 64:128.
            qp = nat.tile([128, nt, 2, d], BF16, tag="qp")
            kp = nat.tile([128, nt, 2, d], BF16, tag="kp")
            hh = nt // 2
            if pair == 0:
                # halved loads, first halves of BOTH heads first, so the
                # first transpose group can start after ~2 trigger slots
                for lo, hi in ((0, hh), (hh, nt)):
                    for src_ap, dst in ((q_ap, qp), (k_ap, kp)):
                        for idx, n in enumerate(st8["heads"]):
                            nc.gpsimd.dma_start(
                                out=dst[:, lo:hi, idx, :], in_=src_ap[n, :, lo:hi]
                            )
            else:
                for idx, n in enumerate(st8["heads"]):
                    nc.gpsimd.dma_start(out=qp[:, :, idx, :], in_=q_ap[n])
                    nc.gpsimd.dma_start(out=kp[:, :, idx, :], in_=k_ap[n])
            vp = nat.tile([128, nt, 2, dv], BF16, tag="vp")
            # ones columns for the softmax-denominator trick
            nc.vector.memset(vp[:, :, :, d : d + 1], 1.0)
            for idx, n in enumerate(st8["heads"]):
                nc.gpsimd.dma_start(out=vp[:, :, idx, 0:d], in_=v_ap[n])
            st8["v"] = vp
            qt = dmaj.tile([128, seq], BF16, tag="qt")
            kt = dmaj.tile([128, seq], BF16, tag="kt")
            for g in range(nt // 4):
                for src, dst in ((qp, qt), (kp, kt)):
                    tp = tp_ps.tile([128, 512], BF16, tag="tp")
                    for u in range(4):
                        t = g * 4 + u
                        nc.tensor.transpose(
                            tp[:, u * 128 : (u + 1) * 128],
                            src[:, t, :, :],
                            ident_bf[:],
                        )
                    nc.vector.tensor_copy(
                        out=dst[:, g * 512 : (g + 1) * 512], in_=tp[:]
                    )
                    if pair == 0:
                        # PE is otherwise DMA-bound here; keep the clock warm
                        pe_filler(3)
            st8["qt"], st8["kt"] = qt, kt
            return st8

        def _exp(s, jb, idx, st):
            """Evacuate one St tile: exact Exp on ScalarE, or the Schraudolph
            bit-trick on VectorE for tiles in SCHRAUD_TILES.  High priority:
            St must evacuate ASAP to release PSUM for the next mm1 quartet,
            ahead of same-engine epilogue work (ot copies / normalize)."""
            with tc.high_priority(offset=30):
                pt = ptp.tile([128, seq], BF16, name="pt", tag="pt")
                if (jb, idx) in SCHRAUD_TILES:
                    nc.vector.tensor_scalar(
                        out=pt[:].bitcast(I16),
                        in0=st[:],
                        scalar1=SCHRAUD_A,
                        scalar2=SCHRAUD_B,
                        op0=TS_MULT,
                        op1=TS_ADD,
                    )
                else:
                    nc.scalar.activation(
                        out=pt[:],
                        in_=st[:],
                        func=mybir.ActivationFunctionType.Exp,
                        scale=SCALE,
                    )
                s["pts"][(jb, idx)] = pt

        def mm1_exp(s, jb):
            """One software-pipelined mm1 step: head A's block jb together
            with head B's block jb-1 (B SHIFTED ONE STEP behind A).

            Head A contracts on PE rows 0:64 (row group h0), head B on
            64:128 (h64).  Emitted interleaved A(c),B(c): consecutive
            instructions target DISJOINT row groups, so the PE streams them
            CONCURRENTLY (~2x issue rate vs same-group runs).  The one-step
            B shift is what makes this robust: B(jb-1)'s PSUM slot was
            released by exp-B(jb-2), a full step ago, so whenever A(jb)
            becomes ready B is ready too and the scheduler bakes the
            quartet adjacently instead of splitting it around mm2 work."""
            qt, kt = s["qt"], s["kt"]
            stA = st_ps.tile([128, seq], F32, name="stA", tag="st")
            stB = None
            if jb > 0:
                stB = st_ps.tile([128, seq], F32, name="stB", tag="st")
            # A's chunks FIRST and adjacent: exp-A(jb) gates the next step's
            # mm1-A via the St PSUM ring (the critical latency chain), so it
            # must start as early as possible.  B(jb-1)'s chunks follow; its
            # exp has a full step of slack.
            names = []
            for c in range(nh):
                names.append(nc.tensor.matmul(
                    stA[:, c * 512 : (c + 1) * 512],
                    kt[0:64, jb * 128 : (jb + 1) * 128],
                    qt[0:64, c * 512 : (c + 1) * 512],
                    start=True,
                    stop=True,
                ).ins.name)
            _exp(s, jb, 0, stA)
            if stB is not None:
                for c in range(nh):
                    names.append(nc.tensor.matmul(
                        stB[:, c * 512 : (c + 1) * 512],
                        kt[64:128, (jb - 1) * 128 : jb * 128],
                        qt[64:128, c * 512 : (c + 1) * 512],
                        start=True,
                        stop=True,
                    ).ins.name)
                quartets.append(tuple(names))  # (A0, A1, B0, B1)
                _exp(s, jb - 1, 1, stB)

        def mm1_tail(s):
            """Head B's last block (jb=nt-1), deferred by the one-step
            shift."""
            qt, kt = s["qt"], s["kt"]
            stB = st_ps.tile([128, seq], F32, name="stB", tag="st")
            for c in range(nh):
                nc.tensor.matmul(
                    stB[:, c * 512 : (c + 1) * 512],
                    kt[64:128, (nt - 1) * 128 : nt * 128],
                    qt[64:128, c * 512 : (c + 1) * 512],
                    start=True,
                    stop=True,
                )
            _exp(s, nt - 1, 1, stB)

        def mm2_unit(s, u):
            """FLIPPED mm2 for one (head, i-quad): stationary = Pt j-block
            [128, 128] (FWL-eligible: full 128 bf16 columns), moving =
            [V(jb) | ones] [128, 65].  The 128-col LDWEIGHTS hides entirely
            behind the previous 65-col stream (HW-measured ~29 ns/matmul
            sustained vs ~216 ns for the V-stationary orientation), and the
            output lands i-MAJOR [128 i, 65], so the whole transpose-back
            epilogue (ot copy + 4 PE transposes + ob stage) disappears.
            Accumulation is ib-outer/jb-inner: start=True clears has_written
            for the WHOLE bank, so each ib region must fully accumulate
            before the next region's start."""
            idx, quad = u // 2, u % 2
            n = s["heads"][idx]
            # [128, 4, 128] fp32 = exactly one PSUM bank; dv slice per ib
            oacc = oa_ps.tile([128, 4, 128], F32, name="oacc", tag="oacc")
            for k in range(4):
                ib = quad * 4 + k
                for jb in range(nt):
                    pt = s["pts"][(jb, idx)]
                    nc.tensor.matmul(
                        oacc[:, k, 0:dv],
                        pt[:, ib * 128 : (ib + 1) * 128],
                        s["v"][:, jb, idx, 0:dv],
                        start=(jb == 0),
                        stop=(jb == nt - 1),
                    )
            if idx not in s["ostage"]:
                s["ostage"][idx] = outp.tile(
                    [128, nt, d], F32, name="ostage", tag="ostage"
                )
            ostage = s["ostage"][idx]
            # out partition p of block ib <-> seq p*8 + ib: ostage[:, ib, :]
            rec = small.tile([128, 4], F32, tag="rec")
            nc.vector.reciprocal(out=rec[:], in_=oacc[:, :, d])
            if quad == 0:
                # i-major normalize fits ScalarE's per-partition scale AP;
                # split quads across engines to balance Sc/DVE load
                for k in range(4):
                    nc.scalar.activation(
                        out=ostage[:, quad * 4 + k, :],
                        in_=oacc[:, k, 0:d],
                        func=mybir.ActivationFunctionType.Copy,
                        scale=rec[:, k : k + 1],
                    )
            else:
                nc.vector.tensor_mul(
                    ostage[:, quad * 4 : (quad + 1) * 4, :],
                    oacc[:, :, 0:d],
                    rec[:, :, None].broadcast_to([128, 4, d]),
                )
            nc.sync.dma_start(
                out=o_ap[:, quad * 4 : (quad + 1) * 4, n * d : (n + 1) * d],
                in_=ostage[:, quad * 4 : (quad + 1) * 4, :],
            )

        # software pipeline: pair p's four mm2 units (A-q0, A-q1, B-q0,
        # B-q1) run during pair p+1's steps 0,2,4,6 — every unit needs ALL
        # of its head's Pt tiles (all 8 j-blocks enter each accumulation).
        slot_at = {}
        for pair in range(heads // 2):
            for u in range(4):
                slot_at.setdefault((pair + 1) * nt + 2 * u, []).append((pair, u))
        states = []
        for pair in range(heads // 2):
            cur = load_and_transpose(pair)
            states.append(cur)
            for jb in range(nt):
                J = pair * nt + jb
                todo = slot_at.pop(J, [])
                # mm2 burst FIRST: PE work between exp(jb-1) and mm1(jb) so
                # the St tiles are free when the mm1 quartet issues.
                for p_, u in todo:
                    mm2_unit(states[p_], u)
                mm1_exp(cur, jb)
                if not todo and J < 7:
                    # keep the PE clock gate warm through the fill phase
                    pe_filler(2)
            mm1_tail(cur)
        for J in sorted(slot_at):
            for p_, u in slot_at[J]:
                mm2_unit(states[p_], u)

    _interleave_quartets(nc, mybir, quartets)
    _split_multi_waits(nc, mybir)
    return nc


def _get_nc():
    if "nc" not in _cache:
        _cache["nc"] = _build_nc()
    return _cache["nc"]


def _run(q, k, v, trace=False):
    from concourse.bass_utils import run_bass_kernel_spmd

    b, heads, h, w, d = 8, 8, 32, 32, 64
    q = np.ascontiguousarray(np.asarray(q, dtype=np.float32))
    k = np.ascontiguousarray(np.asarray(k, dtype=np.float32))
    v = np.ascontiguousarray(np.asarray(v, dtype=np.float32))
    assert q.shape == (b, heads, h, w, d), q.shape

    nc = _get_nc()
    in_maps = [
        {
            "q": q[c].reshape(heads, h * w, d),
            "k": k[c].reshape(heads, h * w, d),
            "v": v[c].reshape(heads, h * w, d),
        }
        for c in range(b)
    ]
    res = run_bass_kernel_spmd(nc, in_maps, core_ids=list(range(b)), trace=trace)
    out = np.stack(
        [res.results[c]["out"].reshape(h, w, heads * d) for c in range(b)]
    )
    return out, res


def kernel(q, k, v):
    out, _ = _run(q, k, v)
    return out



# revision 41
# speedup vs baseline: 1.2507x; 1.2507x over previous
"""Trainium2 Bass kernel for multi-head quadratic spatial attention.

Problem: q,k,v [b=8, heads=8, h=32, w=32, d=64] fp32; full attention over
the 1024-position spatial grid independently per (b, head); output
[b, h, w, heads*d].

Sharding: data-parallel over batch — core c handles b=c (8 heads of
[1024, 64] attention per core), no cross-core communication.

Per-core pipeline (heads processed in PAIRS; matmuls bf16 with fp32 PSUM
accumulation). The PE executes serially on this toolchain, so the design
minimizes streamed columns + instruction count and keeps the HAM clock
gate warm (no transpose-heavy stretches > ~3.4us, dummy-matmul warm-up):
  - p-major seq tiling (seq = p*8 + t); ONE 4D casting DMA per (tensor,
    pair) interleaving the two heads -> 3 gpsimd triggers per pair
  - 40 dummy ident matmuls warm the PE clock gate (1.2 -> 2.4 GHz) while
    the first DMAs land
  - pair-interleaved natural tiles [128, t, 2, d]: one [128,128] PE
    transpose per block yields head A's d-rows on partitions 0:64 and
    B's on 64:128 — the packed pair layout mm1 wants
  - mm1 row-tiled: head A contracts on PE rows 0:64, head B on 64:128
    -> St [128, 1024] fp32 (separate tiles, freed by their own exp)
  - exp on ScalarE (activation Exp); optional per-(jb, head) offload to
    VectorE via the Schraudolph bit-trick (fused tensor_scalar
    mult+add -> int16 == bf16 exp approx) to unload the ScalarE
  - mm2: lhsT = [V | 1] j-chunk [128, 65] bf16, rhs = Pt slices ->
    accumulate PSUM Ot [65, 512] per i-half; row 64 = softmax sums
  - epilogue in bf16: ot copy on VectorE, PE transposes back (FWL), one
    batched reciprocal [128,4] + per-block tensor_scalar normalize into
    fp32 ostage, stores on the sync HWDGE ring
"""

from contextlib import ExitStack

import numpy as np

F32 = None
BF16 = None
I16 = None

_cache = {}

# Schraudolph exp in bf16 bit-space: bf16_bits(exp(s*x)) ~= round(x*A + B)
# A = s * 2^7/ln2, B = 2^7*(127 - sigma), sigma = 0.0430 balances the
# piecewise-linear 2^frac error (max rel err ~3%, mostly cancelling in the
# softmax average).
SCALE = 64.0 ** -0.5
SCHRAUD_A = SCALE * 128.0 / float(np.log(2.0))
SCHRAUD_B = 128.0 * (127.0 - 0.0430)

N_WARM = 10  # dummy matmuls to flip the PE HAM clock gate before real work

# (jb, head-in-pair) St tiles exp'd on VectorE via the Schraudolph bit-trick
# instead of ScalarE's exact exp. Head B's tiles all go to VectorE: exp-A
# (ScalarE, ~1.15us) and exp-B (VectorE, ~1.19us) then run CONCURRENTLY, so
# stA/stB release near-simultaneously and the scheduler can bake the mm1
# quartet as A,B,A,B (adjacent disjoint row groups -> concurrent on the PE).
SCHRAUD_TILES = frozenset((jb, 1) for jb in range(8))


def _imports():
    global F32, BF16, I16
    import concourse.bass as bass
    import concourse.tile as tile
    from concourse import mybir
    from concourse.masks import make_identity

    F32 = mybir.dt.float32
    BF16 = mybir.dt.bfloat16
    I16 = mybir.dt.int16
    return bass, tile, mybir, make_identity


def _interleave_quartets(nc, mybir, quartets):
    """Post-schedule pass: rebake each mm1 quartet as A0,B0,A1,B1 in the PE
    stream (adjacent disjoint row groups stream CONCURRENTLY on the PE).

    The tile scheduler's cost model sees the DVE as backlogged and bakes the
    B matmuls several mm2 slots after the A's, which forfeits row-group
    concurrency.  On hardware the B's DVE wait is long satisfied by then, so
    we permute the PE stream directly: pull each B (with its LDWEIGHTS)
    forward to sit right after its A sibling.  All waits (any engine) on
    semaphores that PE instructions increment are then remapped: a wait for
    "first v PE-increments done" must now cover the same SET of original
    instructions at their new positions, i.e. new_v = prefix count at the
    maximum new position of that set.
    """
    pe_eng = mybir.EngineType.PE

    for f in nc.m.functions:
        for bb in f.blocks:
            insts = bb.instructions
            pe_idx = [i for i, ins in enumerate(insts) if ins.engine == pe_eng]
            if not pe_idx:
                continue
            stream = [insts[i] for i in pe_idx]
            name_pos = {ins.name: p for p, ins in enumerate(stream)}

            # Pair each matmul with ITS ldweights by weights-AP content: the
            # PE reorder window means the baked stream can hoist an ldw
            # several slots ahead, so adjacency is not a safe pairing rule
            # (moving a matmul without its ldw reorders the weight load
            # after the matmul -> wrong weights / wedged device).
            def w_sig(ins):
                tn = type(ins).__name__
                if tn == "InstLdweights":
                    return str(ins.ins[0])
                if tn == "InstMatmult":
                    return str(ins.ins[1])
                return None

            claimed_ldw = set()

            def unit(p):
                sig = w_sig(stream[p])
                for q in range(p - 1, max(-1, p - 8), -1):
                    if (q not in claimed_ldw
                            and type(stream[q]).__name__ == "InstLdweights"
                            and w_sig(stream[q]) == sig):
                        claimed_ldw.add(q)
                        return [q, p]
                return [p]

            # per-sem prefix increment maps
            def upd_map(ins):
                out = {}
                si = ins.sync_info
                if si and si.on_update:
                    for u in si.on_update:
                        if u.sync_type == "semaphore":
                            out[u.id] = out.get(u.id, 0) + (u.update_value or 1)
                return out

            sem_ids = set()
            for ins in stream:
                sem_ids.update(upd_map(ins))
            old_prefix = {s: [0] * (len(stream) + 1) for s in sem_ids}
            for p, ins in enumerate(stream):
                um = upd_map(ins)
                for s in sem_ids:
                    old_prefix[s][p + 1] = old_prefix[s][p] + um.get(s, 0)

            import bisect

            def build_order(active):
                nonlocal claimed_ldw
                claimed_ldw = set()
                order = list(range(len(stream)))
                members = {}  # old position -> quartet key
                for qi, (qa0, qa1, qb0, qb1) in enumerate(quartets):
                    if qi not in active:
                        continue
                    if any(n not in name_pos for n in (qa0, qa1, qb0, qb1)):
                        continue
                    # four matmuls (each with its ldw) contiguous, anchored
                    # at the first matmul's slot (not its possibly hoisted
                    # ldw slot, which would displace unrelated work).
                    units = [unit(name_pos[n]) for n in (qa0, qa1, qb0, qb1)]
                    taken = [t for u in units for t in u]
                    for t in taken:
                        members[t] = qi
                    taken_set = set(taken)
                    anchor = min(order.index(name_pos[n])
                                 for n in (qa0, qa1, qb0, qb1))
                    rest = [x for x in order if x not in taken_set]
                    anchor -= sum(1 for t in taken if order.index(t) < anchor)
                    order = rest[:anchor] + taken + rest[anchor:]
                return order, members

            def target_newpos(w, new_pos_of_old):
                """New wait value for a wait on a PE-tracked sem: the
                original producer instruction's new position (target-
                precise; covering the full displaced set can close
                same-engine ordering cycles)."""
                v = w.wait_value
                if (w.sync_type != "semaphore" or w.id not in sem_ids
                        or v is None or v <= 0 or v > old_prefix[w.id][-1]):
                    return None
                k = bisect.bisect_left(old_prefix[w.id], v)
                return new_pos_of_old[k - 1]

            # Iterate: drop any quartet whose MOVED instructions end up with
            # a same-engine wait whose producer now sits at/after them (an
            # unsatisfiable forward wait on the in-order PE queue).
            active = set(range(len(quartets)))
            while True:
                order, members = build_order(active)
                new_pos_of_old = [0] * len(stream)
                for newp, oldp in enumerate(order):
                    new_pos_of_old[oldp] = newp
                bad = set()
                for oldp, ins in enumerate(stream):
                    si = ins.sync_info
                    if not (si and si.on_wait):
                        continue
                    for w in si.on_wait:
                        tp_ = target_newpos(w, new_pos_of_old)
                        if tp_ is not None and tp_ >= new_pos_of_old[oldp]:
                            bad.add(members.get(oldp, -1))
                if bad - {-1} and bad != {-1}:
                    active -= bad
                    continue
                break

            if order == list(range(len(stream))):
                continue

            new_stream = [stream[i] for i in order]
            new_prefix = {s: [0] * (len(stream) + 1) for s in sem_ids}
            for p, ins in enumerate(new_stream):
                um = upd_map(ins)
                for s in sem_ids:
                    new_prefix[s][p + 1] = new_prefix[s][p] + um.get(s, 0)

            def remap_wait(w):
                tp_ = target_newpos(w, new_pos_of_old)
                if tp_ is not None:
                    w.wait_value = new_prefix[w.id][tp_ + 1]

            for bb2 in f.blocks:
                for ins in bb2.instructions:
                    si = ins.sync_info
                    if si and si.on_wait:
                        for w in si.on_wait:
                            remap_wait(w)

            # write back: permuted PE stream into the same slots
            for slot, ins in zip(pe_idx, new_stream):
                insts[slot] = ins
            bb.instructions = insts


def _split_multi_waits(nc, mybir):
    """Walrus in this container supports only ONE sync-wait per instruction.
    Hoist extra waits onto same-engine InstNoOp's inserted just before."""
    ctr = 0
    for f in nc.m.functions:
        for bb in f.blocks:
            insts = bb.instructions
            if not any(
                i.sync_info and i.sync_info.on_wait and len(i.sync_info.on_wait) > 1
                for i in insts
            ):
                continue
            out = []
            for inst in insts:
                si = inst.sync_info
                waits = list(si.on_wait) if si and si.on_wait else []
                if len(waits) > 1:
                    for w in waits[:-1]:
                        ctr += 1
                        nop = mybir.InstNoOp(
                            name=f"I-wsplit-{ctr}",
                            engine=inst.engine,
                            ins=[],
                            outs=[],
                            sync_info=mybir.SyncInfo(on_wait=[w], on_update=[]),
                        )
                        nc.register_instruction(nop)
                        out.append(nop)
                    si.on_wait = waits[-1:]
                out.append(inst)
            bb.instructions = out


def _build_nc(heads=8, seq=1024, d=64):
    bass, tile, mybir, make_identity = _imports()
    assert heads % 2 == 0 and seq == 1024 and d == 64
    nt = seq // 128          # 8 blocks of 128 positions
    nh = seq // 512          # 2 i-halves of 512
    dv = d + 1
    TS_MULT = mybir.AluOpType.mult
    TS_ADD = mybir.AluOpType.add

    nc = bass.Bass(trn_type="TRN2", target_bir_lowering=False)
    quartets = []  # (A0, B0, A1, B1) matmul names for post-schedule interleave
    q_d = nc.dram_tensor("q", [heads, seq, d], F32, kind="ExternalInput")
    k_d = nc.dram_tensor("k", [heads, seq, d], F32, kind="ExternalInput")
    v_d = nc.dram_tensor("v", [heads, seq, d], F32, kind="ExternalInput")
    o_d = nc.dram_tensor("out", [seq, heads * d], F32, kind="ExternalOutput")

    # p-major: seq = p*nt + t; per-(p, t) HBM runs are 256B contiguous
    q_ap = q_d[:].rearrange("n (p t) d -> n p t d", p=128)
    k_ap = k_d[:].rearrange("n (p t) d -> n p t d", p=128)
    v_ap = v_d[:].rearrange("n (p t) d -> n p t d", p=128)
    o_ap = o_d[:].rearrange("(p t) c -> p t c", p=128)

    with tile.TileContext(nc) as tc, ExitStack() as ctx:
        consts = ctx.enter_context(tc.tile_pool(name="consts", bufs=1))
        nat = ctx.enter_context(tc.tile_pool(name="nat", bufs=2))
        dmaj = ctx.enter_context(tc.tile_pool(name="dmaj", bufs=2))
        ptp = ctx.enter_context(tc.tile_pool(name="ptp", bufs=36))
        outp = ctx.enter_context(tc.tile_pool(name="outp", bufs=3))
        small = ctx.enter_context(tc.tile_pool(name="small", bufs=4))

        # PSUM banks: st 2x2 (0-3) + oacc/ob/warm 2x1 (4-5) + tp 2x1 (6-7)
        # PSUM (8 banks): stA ring 2x2 (4) + stB 1x2 (2) + shared scratch
        # ring 2x1 (2) carrying input-transpose staging, warm filler tiles
        # and the mm2 i-major accumulators (lifetimes interleave cleanly:
        # loads sit at pair boundaries, mm2 units mid-pair).
        st_ps = ctx.enter_context(tc.tile_pool(name="st_ps", bufs=2, space="PSUM"))
        scr_ps = ctx.enter_context(tc.tile_pool(name="scr_ps", bufs=2, space="PSUM"))

        ident_bf = consts.tile([128, 128], BF16)
        make_identity(nc, ident_bf[:])

        # Warm-up / filler matmuls keep the PE HAM clock gate at 2.4 GHz:
        # an idle (or transpose-only) stretch > ~3.4us re-throttles the PE
        # clock to 1.2 GHz for the next several microseconds. wsrc is
        # memset-ready within ~200ns of kernel start. N=512 streams give
        # ~213ns of HAM-counted busy per filler instruction.
        wsrc = consts.tile([128, 512], BF16)
        nc.vector.memset(wsrc[:], 0.25)

        def pe_filler(n):
            # fresh tile per burst: fillers WAR-chain only onto transient
            # transpose tiles, never onto live oacc accumulators
            t = scr_ps.tile([128, 512], F32, tag="scr", name="warm")
            for _ in range(n):
                nc.tensor.matmul(
                    t[:], wsrc[:, 0:128], wsrc[:], start=True, stop=True
                )

        pe_filler(N_WARM)

        def load_and_transpose(pair):
            """DMA pair inputs (bf16 cast, one 4D DMA per tensor) and build
            packed d-major tiles: head A on partitions 0:64, head B on
            64:128 (one [128,128] PE transpose per block)."""
            st8 = {"heads": (2 * pair, 2 * pair + 1), "v": None, "pts": {},
                   "oacc": {}, "ostage": {}}
            # pair-interleaved natural tiles: [..., 2, d] with head A at
            # index 0 and head B at 1, so one [128, 128] PE transpose of a
            # block yields A's d-rows on partitions 0:64 and B's on 64:128.
            qp = nat.tile([128, nt, 2, d], BF16, tag="qp")
            kp = nat.tile([128, nt, 2, d], BF16, tag="kp")
            hh = nt // 2
            if pair == 0:
                # halved loads, first halves of BOTH heads first, so the
                # first transpose group can start after ~2 trigger slots
                for lo, hi in ((0, hh), (hh, nt)):
                    for src_ap, dst in ((q_ap, qp), (k_ap, kp)):
                        for idx, n in enumerate(st8["heads"]):
                            nc.gpsimd.dma_start(
                                out=dst[:, lo:hi, idx, :], in_=src_ap[n, :, lo:hi]
                            )
            else:
                for idx, n in enumerate(st8["heads"]):
                    nc.gpsimd.dma_start(out=qp[:, :, idx, :], in_=q_ap[n])
                    nc.gpsimd.dma_start(out=kp[:, :, idx, :], in_=k_ap[n])
            vp = nat.tile([128, nt, 2, dv], BF16, tag="vp")
            # ones columns for the softmax-denominator trick
            nc.vector.memset(vp[:, :, :, d : d + 1], 1.0)
            for idx, n in enumerate(st8["heads"]):
                nc.gpsimd.dma_start(out=vp[:, :, idx, 0:d], in_=v_ap[n])
            st8["v"] = vp
            qt = dmaj.tile([128, seq], BF16, tag="qt")
            kt = dmaj.tile([128, seq], BF16, tag="kt")
            for g in range(nt // 4):
                for src, dst in ((qp, qt), (kp, kt)):
                    # pad to 2KB so every scr-ring slot is one full bank
                    tp = scr_ps.tile([128, 1024], BF16, tag="scr")
                    for u in range(4):
                        t = g * 4 + u
                        nc.tensor.transpose(
                            tp[:, u * 128 : (u + 1) * 128],
                            src[:, t, :, :],
                            ident_bf[:],
                        )
                    # ScalarE evacuation: the DVE must NOT carry this -- a
                    # DVE-queue copy waiting on PE transposes behind a PE
                    # mm1-B that waits the (1-deep) stB WAR on the DVE
                    # closes a deadlock cycle.  ScalarE's PE waits (stA,
                    # 2-deep ring) are loose, so no cycle can form there.
                    nc.scalar.copy(
                        out=dst[:, g * 512 : (g + 1) * 512], in_=tp[:, 0:512]
                    )
                    if pair == 0:
                        # PE is otherwise DMA-bound here; keep the clock warm
                        pe_filler(3)
            st8["qt"], st8["kt"] = qt, kt
            return st8

        def _exp(s, jb, idx, st):
            """Evacuate one St tile: exact Exp on ScalarE, or the Schraudolph
            bit-trick on VectorE for tiles in SCHRAUD_TILES.  High priority:
            St must evacuate ASAP to release PSUM for the next mm1 quartet,
            ahead of same-engine epilogue work (ot copies / normalize)."""
            with tc.high_priority(offset=30):
                pt = ptp.tile([128, seq], BF16, name="pt", tag="pt")
                if (jb, idx) in SCHRAUD_TILES:
                    nc.vector.tensor_scalar(
                        out=pt[:].bitcast(I16),
                        in0=st[:],
                        scalar1=SCHRAUD_A,
                        scalar2=SCHRAUD_B,
                        op0=TS_MULT,
                        op1=TS_ADD,
                    )
                else:
                    nc.scalar.activation(
                        out=pt[:],
                        in_=st[:],
                        func=mybir.ActivationFunctionType.Exp,
                        scale=SCALE,
                    )
                s["pts"][(jb, idx)] = pt

        def mm1_exp(s, jb):
            """One software-pipelined mm1 step: head A's block jb together
            with head B's block jb-1 (B SHIFTED ONE STEP behind A).

            Head A contracts on PE rows 0:64 (row group h0), head B on
            64:128 (h64).  Emitted interleaved A(c),B(c): consecutive
            instructions target DISJOINT row groups, so the PE streams them
            CONCURRENTLY (~2x issue rate vs same-group runs).  The one-step
            B shift is what makes this robust: B(jb-1)'s PSUM slot was
            released by exp-B(jb-2), a full step ago, so whenever A(jb)
            becomes ready B is ready too and the scheduler bakes the
            quartet adjacently instead of splitting it around mm2 work."""
            qt, kt = s["qt"], s["kt"]
            # shared 3-deep ring (6 PSUM banks): the A,B,A,B allocation
            # cadence lands every stA WAR on an exp TWO steps old and every
            # stB WAR on the PREVIOUS step's ScalarE exp-A -- all mm1 gates
            # point backward with slack, which both dissolves the exp-A
            # latency chain and makes cross-engine deadlock impossible.
            stA = st_ps.tile([128, seq], F32, name="stA", tag="st", bufs=3)
            stB = None
            if jb > 0:
                stB = st_ps.tile([128, seq], F32, name="stB", tag="st", bufs=3)
            # A's chunks FIRST and adjacent: exp-A(jb) gates the next step's
            # mm1-A via the St PSUM ring (the critical latency chain), so it
            # must start as early as possible.  B(jb-1)'s chunks follow; its
            # exp has a full step of slack.
            names = []
            for c in range(nh):
                names.append(nc.tensor.matmul(
                    stA[:, c * 512 : (c + 1) * 512],
                    kt[0:64, jb * 128 : (jb + 1) * 128],
                    qt[0:64, c * 512 : (c + 1) * 512],
                    start=True,
                    stop=True,
                ).ins.name)
            _exp(s, jb, 0, stA)
            if stB is not None:
                for c in range(nh):
                    names.append(nc.tensor.matmul(
                        stB[:, c * 512 : (c + 1) * 512],
                        kt[64:128, (jb - 1) * 128 : jb * 128],
                        qt[64:128, c * 512 : (c + 1) * 512],
                        start=True,
                        stop=True,
                    ).ins.name)
                quartets.append(tuple(names))  # (A0, A1, B0, B1)
                _exp(s, jb - 1, 1, stB)

        def mm1_tail(s):
            """Head B's last block (jb=nt-1), deferred by the one-step
            shift."""
            qt, kt = s["qt"], s["kt"]
            stB = st_ps.tile([128, seq], F32, name="stB", tag="st", bufs=3)
            for c in range(nh):
                nc.tensor.matmul(
                    stB[:, c * 512 : (c + 1) * 512],
                    kt[64:128, (nt - 1) * 128 : nt * 128],
                    qt[64:128, c * 512 : (c + 1) * 512],
                    start=True,
                    stop=True,
                )
            _exp(s, nt - 1, 1, stB)

        def mm2_unit(s, u):
            """FLIPPED mm2 for one (head, i-quad): stationary = Pt j-block
            [128, 128] (FWL-eligible: full 128 bf16 columns), moving =
            [V(jb) | ones] [128, 65].  The 128-col LDWEIGHTS hides entirely
            behind the previous 65-col stream (HW-measured ~29 ns/matmul
            sustained vs ~216 ns for the V-stationary orientation), and the
            output lands i-MAJOR [128 i, 65], so the whole transpose-back
            epilogue (ot copy + 4 PE transposes + ob stage) disappears.
            Accumulation is ib-outer/jb-inner: start=True clears has_written
            for the WHOLE bank, so each ib region must fully accumulate
            before the next region's start."""
            idx, quad, half = u // 4, (u // 2) % 2, u % 2
            n = s["heads"][idx]
            # [128, 4, 128] fp32 = exactly one PSUM bank; dv slice per ib.
            # Each (head, quad) runs as TWO half-units of 16 matmuls on
            # consecutive steps: one ~0.5us mm2 burst every step keeps the
            # PE dense enough that the HAM activity monitor stays at K=8/8.
            if half == 0:
                s["oacc"][(idx, quad)] = scr_ps.tile(
                    [128, 4, 128], F32, name="oacc", tag="scr"
                )
            oacc = s["oacc"][(idx, quad)]
            for k in range(2 * half, 2 * half + 2):
                ib = quad * 4 + k
                for jb in range(nt):
                    pt = s["pts"][(jb, idx)]
                    nc.tensor.matmul(
                        oacc[:, k, 0:dv],
                        pt[:, ib * 128 : (ib + 1) * 128],
                        s["v"][:, jb, idx, 0:dv],
                        start=(jb == 0),
                        stop=(jb == nt - 1),
                    )
            if half == 0:
                return
            if idx not in s["ostage"]:
                s["ostage"][idx] = outp.tile(
                    [128, nt, d], F32, name="ostage", tag="ostage"
                )
            ostage = s["ostage"][idx]
            # out partition p of block ib <-> seq p*8 + ib: ostage[:, ib, :]
            rec = small.tile([128, 4], F32, tag="rec")
            nc.vector.reciprocal(out=rec[:], in_=oacc[:, :, d])
            # all-DVE normalize: ScalarE's queue must stay trivially
            # forward-progressing (only exp/copies waiting on earlier PE
            # work); an ACT normalize waiting on a future mm2 stop can
            # close a cross-engine ordering cycle with the mm1 WAR gates.
            nc.vector.tensor_mul(
                ostage[:, quad * 4 : (quad + 1) * 4, :],
                oacc[:, :, 0:d],
                rec[:, :, None].broadcast_to([128, 4, d]),
            )
            nc.sync.dma_start(
                out=o_ap[:, quad * 4 : (quad + 1) * 4, n * d : (n + 1) * d],
                in_=ostage[:, quad * 4 : (quad + 1) * 4, :],
            )

        # software pipeline: pair p's eight mm2 half-units (A-q0a, A-q0b,
        # A-q1a, ... B-q1b) run during pair p+1's steps 0..7 — every unit
        # needs ALL of its head's Pt tiles (all 8 j-blocks enter each
        # accumulation), which exist once pair p's exps have drained.
        slot_at = {}
        for pair in range(heads // 2):
            for u in range(8):
                slot_at.setdefault((pair + 1) * nt + u, []).append((pair, u))
        states = []
        for pair in range(heads // 2):
            cur = load_and_transpose(pair)
            states.append(cur)
            for jb in range(nt):
                J = pair * nt + jb
                todo = slot_at.pop(J, [])
                # mm2 burst FIRST: PE work between exp(jb-1) and mm1(jb) so
                # the St tiles are free when the mm1 quartet issues.
                for p_, u in todo:
                    mm2_unit(states[p_], u)
                mm1_exp(cur, jb)
                if not todo and J < 7:
                    # keep the PE clock gate warm through the fill phase
                    pe_filler(2)
            mm1_tail(cur)
        for J in sorted(slot_at):
            for p_, u in slot_at[J]:
                mm2_unit(states[p_], u)

    # NOTE: a post-schedule PE-stream interleave pass was tried here (rebake
    # mm1 quartets adjacently); with the 3-deep St ring the exp latency chain
    # it addressed is gone, and reordering a wait-subsumption-optimized
    # stream is unsound without the full dep graph (races/deadlocks).
    _split_multi_waits(nc, mybir)
    return nc


def _get_nc():
    if "nc" not in _cache:
        _cache["nc"] = _build_nc()
    return _cache["nc"]


def _run(q, k, v, trace=False):
    from concourse.bass_utils import run_bass_kernel_spmd

    b, heads, h, w, d = 8, 8, 32, 32, 64
    q = np.ascontiguousarray(np.asarray(q, dtype=np.float32))
    k = np.ascontiguousarray(np.asarray(k, dtype=np.float32))
    v = np.ascontiguousarray(np.asarray(v, dtype=np.float32))
    assert q.shape == (b, heads, h, w, d), q.shape

    nc = _get_nc()
    in_maps = [
        {
            "q": q[c].reshape(heads, h * w, d),
            "k": k[c].reshape(heads, h * w, d),
            "v": v[c].reshape(heads, h * w, d),
        }
        for c in range(b)
    ]
    res = run_bass_kernel_spmd(nc, in_maps, core_ids=list(range(b)), trace=trace)
    out = np.stack(
        [res.results[c]["out"].reshape(h, w, heads * d) for c in range(b)]
    )
    return out, res


def kernel(q, k, v):
    out, _ = _run(q, k, v)
    return out



# revision 42
# speedup vs baseline: 1.3878x; 1.1096x over previous
"""Trainium2 Bass kernel for multi-head quadratic spatial attention.

Problem: q,k,v [b=8, heads=8, h=32, w=32, d=64] fp32; full attention over
the 1024-position spatial grid independently per (b, head); output
[b, h, w, heads*d].

Sharding: data-parallel over batch — core c handles b=c (8 heads of
[1024, 64] attention per core), no cross-core communication.

Per-core pipeline (heads processed in PAIRS; matmuls bf16 with fp32 PSUM
accumulation). The PE executes serially on this toolchain, so the design
minimizes streamed columns + instruction count and keeps the HAM clock
gate warm (no transpose-heavy stretches > ~3.4us, dummy-matmul warm-up):
  - p-major seq tiling (seq = p*8 + t); ONE 4D casting DMA per (tensor,
    pair) interleaving the two heads -> 3 gpsimd triggers per pair
  - 40 dummy ident matmuls warm the PE clock gate (1.2 -> 2.4 GHz) while
    the first DMAs land
  - pair-interleaved natural tiles [128, t, 2, d]: one [128,128] PE
    transpose per block yields head A's d-rows on partitions 0:64 and
    B's on 64:128 — the packed pair layout mm1 wants
  - mm1 row-tiled: head A contracts on PE rows 0:64, head B on 64:128
    -> St [128, 1024] fp32 (separate tiles, freed by their own exp)
  - exp on ScalarE (activation Exp); optional per-(jb, head) offload to
    VectorE via the Schraudolph bit-trick (fused tensor_scalar
    mult+add -> int16 == bf16 exp approx) to unload the ScalarE
  - mm2: lhsT = [V | 1] j-chunk [128, 65] bf16, rhs = Pt slices ->
    accumulate PSUM Ot [65, 512] per i-half; row 64 = softmax sums
  - epilogue in bf16: ot copy on VectorE, PE transposes back (FWL), one
    batched reciprocal [128,4] + per-block tensor_scalar normalize into
    fp32 ostage, stores on the sync HWDGE ring
"""

from contextlib import ExitStack

import numpy as np

F32 = None
BF16 = None
I16 = None

_cache = {}

# Schraudolph exp in bf16 bit-space: bf16_bits(exp(s*x)) ~= round(x*A + B)
# A = s * 2^7/ln2, B = 2^7*(127 - sigma), sigma = 0.0430 balances the
# piecewise-linear 2^frac error (max rel err ~3%, mostly cancelling in the
# softmax average).
SCALE = 64.0 ** -0.5
SCHRAUD_A = SCALE * 128.0 / float(np.log(2.0))
SCHRAUD_B = 128.0 * (127.0 - 0.0430)

N_WARM = 6  # dummy matmuls to flip the PE HAM clock gate before real work

# (jb, head-in-pair) St tiles exp'd on VectorE via the Schraudolph bit-trick
# instead of ScalarE's exact exp. Head B's tiles all go to VectorE: exp-A
# (ScalarE, ~1.15us) and exp-B (VectorE, ~1.19us) then run CONCURRENTLY, so
# stA/stB release near-simultaneously and the scheduler can bake the mm1
# quartet as A,B,A,B (adjacent disjoint row groups -> concurrent on the PE).
SCHRAUD_TILES = frozenset((jb, 1) for jb in range(8))


def _imports():
    global F32, BF16, I16
    import concourse.bass as bass
    import concourse.tile as tile
    from concourse import mybir
    from concourse.masks import make_identity

    F32 = mybir.dt.float32
    BF16 = mybir.dt.bfloat16
    I16 = mybir.dt.int16
    return bass, tile, mybir, make_identity


def _interleave_quartets(nc, mybir, quartets):
    """Post-schedule pass: rebake each mm1 quartet as A0,B0,A1,B1 in the PE
    stream (adjacent disjoint row groups stream CONCURRENTLY on the PE).

    The tile scheduler's cost model sees the DVE as backlogged and bakes the
    B matmuls several mm2 slots after the A's, which forfeits row-group
    concurrency.  On hardware the B's DVE wait is long satisfied by then, so
    we permute the PE stream directly: pull each B (with its LDWEIGHTS)
    forward to sit right after its A sibling.  All waits (any engine) on
    semaphores that PE instructions increment are then remapped: a wait for
    "first v PE-increments done" must now cover the same SET of original
    instructions at their new positions, i.e. new_v = prefix count at the
    maximum new position of that set.
    """
    pe_eng = mybir.EngineType.PE

    for f in nc.m.functions:
        for bb in f.blocks:
            insts = bb.instructions
            pe_idx = [i for i, ins in enumerate(insts) if ins.engine == pe_eng]
            if not pe_idx:
                continue
            stream = [insts[i] for i in pe_idx]
            name_pos = {ins.name: p for p, ins in enumerate(stream)}

            # Pair each matmul with ITS ldweights by weights-AP content: the
            # PE reorder window means the baked stream can hoist an ldw
            # several slots ahead, so adjacency is not a safe pairing rule
            # (moving a matmul without its ldw reorders the weight load
            # after the matmul -> wrong weights / wedged device).
            def w_sig(ins):
                tn = type(ins).__name__
                if tn == "InstLdweights":
                    return str(ins.ins[0])
                if tn == "InstMatmult":
                    return str(ins.ins[1])
                return None

            claimed_ldw = set()

            def unit(p):
                sig = w_sig(stream[p])
                for q in range(p - 1, max(-1, p - 8), -1):
                    if (q not in claimed_ldw
                            and type(stream[q]).__name__ == "InstLdweights"
                            and w_sig(stream[q]) == sig):
                        claimed_ldw.add(q)
                        return [q, p]
                return [p]

            # per-sem prefix increment maps
            def upd_map(ins):
                out = {}
                si = ins.sync_info
                if si and si.on_update:
                    for u in si.on_update:
                        if u.sync_type == "semaphore":
                            out[u.id] = out.get(u.id, 0) + (u.update_value or 1)
                return out

            sem_ids = set()
            for ins in stream:
                sem_ids.update(upd_map(ins))
            old_prefix = {s: [0] * (len(stream) + 1) for s in sem_ids}
            for p, ins in enumerate(stream):
                um = upd_map(ins)
                for s in sem_ids:
                    old_prefix[s][p + 1] = old_prefix[s][p] + um.get(s, 0)

            import bisect

            def build_order(active):
                nonlocal claimed_ldw
                claimed_ldw = set()
                order = list(range(len(stream)))
                members = {}  # old position -> quartet key
                for qi, (qa0, qa1, qb0, qb1) in enumerate(quartets):
                    if qi not in active:
                        continue
                    if any(n not in name_pos for n in (qa0, qa1, qb0, qb1)):
                        continue
                    # four matmuls (each with its ldw) contiguous, anchored
                    # at the first matmul's slot (not its possibly hoisted
                    # ldw slot, which would displace unrelated work).
                    units = [unit(name_pos[n]) for n in (qa0, qa1, qb0, qb1)]
                    taken = [t for u in units for t in u]
                    for t in taken:
                        members[t] = qi
                    taken_set = set(taken)
                    anchor = min(order.index(name_pos[n])
                                 for n in (qa0, qa1, qb0, qb1))
                    rest = [x for x in order if x not in taken_set]
                    anchor -= sum(1 for t in taken if order.index(t) < anchor)
                    order = rest[:anchor] + taken + rest[anchor:]
                return order, members

            def target_newpos(w, new_pos_of_old):
                """New wait value for a wait on a PE-tracked sem: the
                original producer instruction's new position (target-
                precise; covering the full displaced set can close
                same-engine ordering cycles)."""
                v = w.wait_value
                if (w.sync_type != "semaphore" or w.id not in sem_ids
                        or v is None or v <= 0 or v > old_prefix[w.id][-1]):
                    return None
                k = bisect.bisect_left(old_prefix[w.id], v)
                return new_pos_of_old[k - 1]

            # Iterate: drop any quartet whose MOVED instructions end up with
            # a same-engine wait whose producer now sits at/after them (an
            # unsatisfiable forward wait on the in-order PE queue).
            active = set(range(len(quartets)))
            while True:
                order, members = build_order(active)
                new_pos_of_old = [0] * len(stream)
                for newp, oldp in enumerate(order):
                    new_pos_of_old[oldp] = newp
                bad = set()
                for oldp, ins in enumerate(stream):
                    si = ins.sync_info
                    if not (si and si.on_wait):
                        continue
                    for w in si.on_wait:
                        tp_ = target_newpos(w, new_pos_of_old)
                        if tp_ is not None and tp_ >= new_pos_of_old[oldp]:
                            bad.add(members.get(oldp, -1))
                if bad - {-1} and bad != {-1}:
                    active -= bad
                    continue
                break

            if order == list(range(len(stream))):
                continue

            new_stream = [stream[i] for i in order]
            new_prefix = {s: [0] * (len(stream) + 1) for s in sem_ids}
            for p, ins in enumerate(new_stream):
                um = upd_map(ins)
                for s in sem_ids:
                    new_prefix[s][p + 1] = new_prefix[s][p] + um.get(s, 0)

            def remap_wait(w):
                tp_ = target_newpos(w, new_pos_of_old)
                if tp_ is not None:
                    w.wait_value = new_prefix[w.id][tp_ + 1]

            for bb2 in f.blocks:
                for ins in bb2.instructions:
                    si = ins.sync_info
                    if si and si.on_wait:
                        for w in si.on_wait:
                            remap_wait(w)

            # write back: permuted PE stream into the same slots
            for slot, ins in zip(pe_idx, new_stream):
                insts[slot] = ins
            bb.instructions = insts


def _split_multi_waits(nc, mybir):
    """Walrus in this container supports only ONE sync-wait per instruction.
    Hoist extra waits onto same-engine InstNoOp's inserted just before."""
    ctr = 0
    for f in nc.m.functions:
        for bb in f.blocks:
            insts = bb.instructions
            if not any(
                i.sync_info and i.sync_info.on_wait and len(i.sync_info.on_wait) > 1
                for i in insts
            ):
                continue
            out = []
            for inst in insts:
                si = inst.sync_info
                waits = list(si.on_wait) if si and si.on_wait else []
                if len(waits) > 1:
                    for w in waits[:-1]:
                        ctr += 1
                        nop = mybir.InstNoOp(
                            name=f"I-wsplit-{ctr}",
                            engine=inst.engine,
                            ins=[],
                            outs=[],
                            sync_info=mybir.SyncInfo(on_wait=[w], on_update=[]),
                        )
                        nc.register_instruction(nop)
                        out.append(nop)
                    si.on_wait = waits[-1:]
                out.append(inst)
            bb.instructions = out


def _build_nc(heads=8, seq=1024, d=64):
    bass, tile, mybir, make_identity = _imports()
    assert heads % 2 == 0 and seq == 1024 and d == 64
    nt = seq // 128          # 8 blocks of 128 positions
    nh = seq // 512          # 2 i-halves of 512
    dv = d + 1
    TS_MULT = mybir.AluOpType.mult
    TS_ADD = mybir.AluOpType.add

    nc = bass.Bass(trn_type="TRN2", target_bir_lowering=False)
    quartets = []  # (A0, B0, A1, B1) matmul names for post-schedule interleave
    q_d = nc.dram_tensor("q", [heads, seq, d], F32, kind="ExternalInput")
    k_d = nc.dram_tensor("k", [heads, seq, d], F32, kind="ExternalInput")
    v_d = nc.dram_tensor("v", [heads, seq, d], F32, kind="ExternalInput")
    o_d = nc.dram_tensor("out", [seq, heads * d], F32, kind="ExternalOutput")

    # p-major: seq = p*nt + t; per-(p, t) HBM runs are 256B contiguous
    q_ap = q_d[:].rearrange("n (p t) d -> n p t d", p=128)
    k_ap = k_d[:].rearrange("n (p t) d -> n p t d", p=128)
    v_ap = v_d[:].rearrange("n (p t) d -> n p t d", p=128)
    o_ap = o_d[:].rearrange("(p t) c -> p t c", p=128)

    with tile.TileContext(nc) as tc, ExitStack() as ctx:
        consts = ctx.enter_context(tc.tile_pool(name="consts", bufs=1))
        nat = ctx.enter_context(tc.tile_pool(name="nat", bufs=2))
        dmaj = ctx.enter_context(tc.tile_pool(name="dmaj", bufs=2))
        ptp = ctx.enter_context(tc.tile_pool(name="ptp", bufs=36))
        outp = ctx.enter_context(tc.tile_pool(name="outp", bufs=3))
        small = ctx.enter_context(tc.tile_pool(name="small", bufs=4))

        # PSUM banks: st 2x2 (0-3) + oacc/ob/warm 2x1 (4-5) + tp 2x1 (6-7)
        # PSUM (8 banks): stA ring 2x2 (4) + stB 1x2 (2) + shared scratch
        # ring 2x1 (2) carrying input-transpose staging, warm filler tiles
        # and the mm2 i-major accumulators (lifetimes interleave cleanly:
        # loads sit at pair boundaries, mm2 units mid-pair).
        st_ps = ctx.enter_context(tc.tile_pool(name="st_ps", bufs=2, space="PSUM"))
        scr_ps = ctx.enter_context(tc.tile_pool(name="scr_ps", bufs=2, space="PSUM"))

        ident_bf = consts.tile([128, 128], BF16)
        make_identity(nc, ident_bf[:])

        # Warm-up / filler matmuls keep the PE HAM clock gate at 2.4 GHz:
        # an idle (or transpose-only) stretch > ~3.4us re-throttles the PE
        # clock to 1.2 GHz for the next several microseconds. wsrc is
        # memset-ready within ~200ns of kernel start. N=512 streams give
        # ~213ns of HAM-counted busy per filler instruction.
        wsrc = consts.tile([128, 512], BF16)
        nc.vector.memset(wsrc[:], 0.25)

        def pe_filler(n):
            # fresh tile per burst: fillers WAR-chain only onto transient
            # transpose tiles, never onto live oacc accumulators
            t = scr_ps.tile([128, 512], F32, tag="scr", name="warm")
            for _ in range(n):
                nc.tensor.matmul(
                    t[:], wsrc[:, 0:128], wsrc[:], start=True, stop=True
                )

        pe_filler(N_WARM)

        def load_and_transpose(pair):
            """DMA pair inputs (bf16 cast, one 4D DMA per tensor) and build
            packed d-major tiles: head A on partitions 0:64, head B on
            64:128 (one [128,128] PE transpose per block)."""
            st8 = {"heads": (2 * pair, 2 * pair + 1), "v": None, "pts": {},
                   "oacc": {}, "ostage": {}}
            # pair-interleaved natural tiles: [..., 2, d] with head A at
            # index 0 and head B at 1, so one [128, 128] PE transpose of a
            # block yields A's d-rows on partitions 0:64 and B's on 64:128.
            qp = nat.tile([128, nt, 2, d], BF16, tag="qp")
            kp = nat.tile([128, nt, 2, d], BF16, tag="kp")
            hh = nt // 2
            if pair == 0:
                # halved loads, first halves of BOTH heads first, so the
                # first transpose group can start after ~2 trigger slots
                for lo, hi in ((0, hh), (hh, nt)):
                    for src_ap, dst in ((q_ap, qp), (k_ap, kp)):
                        for idx, n in enumerate(st8["heads"]):
                            nc.gpsimd.dma_start(
                                out=dst[:, lo:hi, idx, :], in_=src_ap[n, :, lo:hi]
                            )
            else:
                for idx, n in enumerate(st8["heads"]):
                    nc.gpsimd.dma_start(out=qp[:, :, idx, :], in_=q_ap[n])
                    nc.gpsimd.dma_start(out=kp[:, :, idx, :], in_=k_ap[n])
            vp = nat.tile([128, nt, 2, dv], BF16, tag="vp")
            # ones columns for the softmax-denominator trick
            nc.vector.memset(vp[:, :, :, d : d + 1], 1.0)
            for idx, n in enumerate(st8["heads"]):
                nc.gpsimd.dma_start(out=vp[:, :, idx, 0:d], in_=v_ap[n])
            st8["v"] = vp
            qt = dmaj.tile([128, seq], BF16, tag="qt")
            kt = dmaj.tile([128, seq], BF16, tag="kt")
            for g in range(nt // 4):
                for src, dst in ((qp, qt), (kp, kt)):
                    # pad to 2KB so every scr-ring slot is one full bank
                    tp = scr_ps.tile([128, 1024], BF16, tag="scr")
                    for u in range(4):
                        t = g * 4 + u
                        nc.tensor.transpose(
                            tp[:, u * 128 : (u + 1) * 128],
                            src[:, t, :, :],
                            ident_bf[:],
                        )
                    # ScalarE evacuation: the DVE must NOT carry this -- a
                    # DVE-queue copy waiting on PE transposes behind a PE
                    # mm1-B that waits the (1-deep) stB WAR on the DVE
                    # closes a deadlock cycle.  ScalarE's PE waits (stA,
                    # 2-deep ring) are loose, so no cycle can form there.
                    nc.scalar.copy(
                        out=dst[:, g * 512 : (g + 1) * 512], in_=tp[:, 0:512]
                    )
                    if pair == 0:
                        # PE is otherwise DMA-bound here; keep the clock warm
                        pe_filler(3)
            st8["qt"], st8["kt"] = qt, kt
            return st8

        def _exp(s, jb, idx, st):
            """Evacuate one St tile: exact Exp on ScalarE, or the Schraudolph
            bit-trick on VectorE for tiles in SCHRAUD_TILES.  High priority:
            St must evacuate ASAP to release PSUM for the next mm1 quartet,
            ahead of same-engine epilogue work (ot copies / normalize)."""
            with tc.high_priority(offset=30):
                pt = ptp.tile([128, seq], BF16, name="pt", tag="pt")
                if (jb, idx) in SCHRAUD_TILES:
                    nc.vector.tensor_scalar(
                        out=pt[:].bitcast(I16),
                        in0=st[:],
                        scalar1=SCHRAUD_A,
                        scalar2=SCHRAUD_B,
                        op0=TS_MULT,
                        op1=TS_ADD,
                    )
                else:
                    nc.scalar.activation(
                        out=pt[:],
                        in_=st[:],
                        func=mybir.ActivationFunctionType.Exp,
                        scale=SCALE,
                    )
                s["pts"][(jb, idx)] = pt

        def mm1_exp(s, jb):
            """One software-pipelined mm1 step: head A's block jb together
            with head B's block jb-1 (B SHIFTED ONE STEP behind A).

            Head A contracts on PE rows 0:64 (row group h0), head B on
            64:128 (h64).  Emitted interleaved A(c),B(c): consecutive
            instructions target DISJOINT row groups, so the PE streams them
            CONCURRENTLY (~2x issue rate vs same-group runs).  The one-step
            B shift is what makes this robust: B(jb-1)'s PSUM slot was
            released by exp-B(jb-2), a full step ago, so whenever A(jb)
            becomes ready B is ready too and the scheduler bakes the
            quartet adjacently instead of splitting it around mm2 work."""
            qt, kt = s["qt"], s["kt"]
            # shared 3-deep ring (6 PSUM banks): the A,B,A,B allocation
            # cadence lands every stA WAR on an exp TWO steps old and every
            # stB WAR on the PREVIOUS step's ScalarE exp-A -- all mm1 gates
            # point backward with slack, which both dissolves the exp-A
            # latency chain and makes cross-engine deadlock impossible.
            stA = st_ps.tile([128, seq], F32, name="stA", tag="st", bufs=3)
            stB = None
            if jb > 0:
                stB = st_ps.tile([128, seq], F32, name="stB", tag="st", bufs=3)
            # ~107ns of HAM food per step: the PE runs ~0.1us under the
            # engine-paced step, and without it the activity monitor sees
            # enough idle to duty-throttle the clock (K=4/8) mid-kernel.
            # Writing into stA is free: the real mm1's start=True clears
            # has_written and overwrites.
            nc.tensor.matmul(
                stA[:, 0:256], wsrc[:, 0:128], wsrc[:, 0:256],
                start=True, stop=True,
            )
            # A's chunks FIRST and adjacent: exp-A(jb) gates the next step's
            # mm1-A via the St PSUM ring (the critical latency chain), so it
            # must start as early as possible.  B(jb-1)'s chunks follow; its
            # exp has a full step of slack.
            names = []
            for c in range(nh):
                names.append(nc.tensor.matmul(
                    stA[:, c * 512 : (c + 1) * 512],
                    kt[0:64, jb * 128 : (jb + 1) * 128],
                    qt[0:64, c * 512 : (c + 1) * 512],
                    start=True,
                    stop=True,
                ).ins.name)
            _exp(s, jb, 0, stA)
            if stB is not None:
                for c in range(nh):
                    names.append(nc.tensor.matmul(
                        stB[:, c * 512 : (c + 1) * 512],
                        kt[64:128, (jb - 1) * 128 : jb * 128],
                        qt[64:128, c * 512 : (c + 1) * 512],
                        start=True,
                        stop=True,
                    ).ins.name)
                quartets.append(tuple(names))  # (A0, A1, B0, B1)
                _exp(s, jb - 1, 1, stB)

        def mm1_tail(s):
            """Head B's last block (jb=nt-1), deferred by the one-step
            shift."""
            qt, kt = s["qt"], s["kt"]
            stB = st_ps.tile([128, seq], F32, name="stB", tag="st", bufs=3)
            for c in range(nh):
                nc.tensor.matmul(
                    stB[:, c * 512 : (c + 1) * 512],
                    kt[64:128, (nt - 1) * 128 : nt * 128],
                    qt[64:128, c * 512 : (c + 1) * 512],
                    start=True,
                    stop=True,
                )
            _exp(s, nt - 1, 1, stB)

        def mm2_unit(s, u):
            """FLIPPED mm2 for one (head, i-quad): stationary = Pt j-block
            [128, 128] (FWL-eligible: full 128 bf16 columns), moving =
            [V(jb) | ones] [128, 65].  The 128-col LDWEIGHTS hides entirely
            behind the previous 65-col stream (HW-measured ~29 ns/matmul
            sustained vs ~216 ns for the V-stationary orientation), and the
            output lands i-MAJOR [128 i, 65], so the whole transpose-back
            epilogue (ot copy + 4 PE transposes + ob stage) disappears.
            Accumulation is ib-outer/jb-inner: start=True clears has_written
            for the WHOLE bank, so each ib region must fully accumulate
            before the next region's start."""
            idx, quad, half = u // 4, (u // 2) % 2, u % 2
            n = s["heads"][idx]
            # [128, 4, 128] fp32 = exactly one PSUM bank; dv slice per ib.
            # Each (head, quad) runs as TWO half-units of 16 matmuls on
            # consecutive steps: one ~0.5us mm2 burst every step keeps the
            # PE dense enough that the HAM activity monitor stays at K=8/8.
            if half == 0:
                s["oacc"][(idx, quad)] = scr_ps.tile(
                    [128, 4, 128], F32, name="oacc", tag="scr"
                )
            oacc = s["oacc"][(idx, quad)]
            for k in range(2 * half, 2 * half + 2):
                ib = quad * 4 + k
                for jb in range(nt):
                    pt = s["pts"][(jb, idx)]
                    nc.tensor.matmul(
                        oacc[:, k, 0:dv],
                        pt[:, ib * 128 : (ib + 1) * 128],
                        s["v"][:, jb, idx, 0:dv],
                        start=(jb == 0),
                        stop=(jb == nt - 1),
                    )
            if half == 0:
                return
            if idx not in s["ostage"]:
                s["ostage"][idx] = outp.tile(
                    [128, nt, d], F32, name="ostage", tag="ostage"
                )
            ostage = s["ostage"][idx]
            # out partition p of block ib <-> seq p*8 + ib: ostage[:, ib, :]
            rec = small.tile([128, 4], F32, tag="rec")
            nc.vector.reciprocal(out=rec[:], in_=oacc[:, :, d])
            # all-DVE normalize: ScalarE's queue must stay trivially
            # forward-progressing (only exp/copies waiting on earlier PE
            # work); an ACT normalize waiting on a future mm2 stop can
            # close a cross-engine ordering cycle with the mm1 WAR gates.
            nc.vector.tensor_mul(
                ostage[:, quad * 4 : (quad + 1) * 4, :],
                oacc[:, :, 0:d],
                rec[:, :, None].broadcast_to([128, 4, d]),
            )
            nc.sync.dma_start(
                out=o_ap[:, quad * 4 : (quad + 1) * 4, n * d : (n + 1) * d],
                in_=ostage[:, quad * 4 : (quad + 1) * 4, :],
            )

        # software pipeline: pair p's eight mm2 half-units (A-q0a, A-q0b,
        # A-q1a, ... B-q1b) run during pair p+1's steps 0..7 — every unit
        # needs ALL of its head's Pt tiles (all 8 j-blocks enter each
        # accumulation), which exist once pair p's exps have drained.
        slot_at = {}
        for pair in range(heads // 2):
            for u in range(8):
                slot_at.setdefault((pair + 1) * nt + u, []).append((pair, u))
        states = []
        for pair in range(heads // 2):
            cur = load_and_transpose(pair)
            states.append(cur)
            for jb in range(nt):
                J = pair * nt + jb
                todo = slot_at.pop(J, [])
                # mm2 burst FIRST: PE work between exp(jb-1) and mm1(jb) so
                # the St tiles are free when the mm1 quartet issues.
                for p_, u in todo:
                    mm2_unit(states[p_], u)
                mm1_exp(cur, jb)
                if not todo and J < 7:
                    # keep the PE clock gate warm through the fill phase
                    pe_filler(2)
            mm1_tail(cur)
        for J in sorted(slot_at):
            for p_, u in slot_at[J]:
                mm2_unit(states[p_], u)

    # NOTE: a post-schedule PE-stream interleave pass was tried here (rebake
    # mm1 quartets adjacently); with the 3-deep St ring the exp latency chain
    # it addressed is gone, and reordering a wait-subsumption-optimized
    # stream is unsound without the full dep graph (races/deadlocks).
    _split_multi_waits(nc, mybir)
    return nc


def _get_nc():
    if "nc" not in _cache:
        _cache["nc"] = _build_nc()
    return _cache["nc"]


def _run(q, k, v, trace=False):
    from concourse.bass_utils import run_bass_kernel_spmd

    b, heads, h, w, d = 8, 8, 32, 32, 64
    q = np.ascontiguousarray(np.asarray(q, dtype=np.float32))
    k = np.ascontiguousarray(np.asarray(k, dtype=np.float32))
    v = np.ascontiguousarray(np.asarray(v, dtype=np.float32))
    assert q.shape == (b, heads, h, w, d), q.shape

    nc = _get_nc()
    in_maps = [
        {
            "q": q[c].reshape(heads, h * w, d),
            "k": k[c].reshape(heads, h * w, d),
            "v": v[c].reshape(heads, h * w, d),
        }
        for c in range(b)
    ]
    res = run_bass_kernel_spmd(nc, in_maps, core_ids=list(range(b)), trace=trace)
    out = np.stack(
        [res.results[c]["out"].reshape(h, w, heads * d) for c in range(b)]
    )
    return out, res


def kernel(q, k, v):
    out, _ = _run(q, k, v)
    return out

